# revision 2
# baseline (speedup 1.0000x reference)
"""Trainium2 Bass kernel for nn_CFLiner (Clifford-algebra linear layer).

Math: out[b,k,c] = sum_{i,j} input[b,i] * weight[k,j] * M[i,j,c] + bias[c]
where M[i,j,c] = rs[i,j] * [c == i^j] is the Cl(8,0) blade-product tensor.
Since rk[i,j] = i XOR j, folding weight into M is a signed gather:
    kic[i, (k,c)] = weight[k, i^c] * rs[i, i^c]
and the heavy op is a dense matmul  out[b, (k,c)] = input[b,:] @ kic[:, (k,c)].

Strategy: pure data parallelism over 8 NeuronCores (batch 16384 -> 2048/core).
Host prepares (outside HW timing): A^T shards + kic in bf16. Device runs the
[2048,256]x[256,4096] matmul in bf16 on the PE, drains PSUM via ScalarE/VectorE,
DMAs the f32 output (32MB/core) back -- memory-bound on the output write.
"""

import sys
import numpy as np

for _p in ("/opt/trn_rl_repo",):
    if _p not in sys.path:
        sys.path.append(_p)

import ml_dtypes

BATCH = 16384
S = 256          # blade dimension
K = 16           # out channels
NCORES = 8
BPC = BATCH // NCORES   # 2048 rows per core
KC = K * S              # 4096 output columns (k*256 + c)

_BF16 = ml_dtypes.bfloat16

# ---------------------------------------------------------------------------
# Compile-time constant tables (blade-product structure of Cl(8,0))
# ---------------------------------------------------------------------------
_tables_cache = {}


def _blade_combine(a, b):
    if a == 0:
        return b, 1
    if b == 0:
        return a, 1
    c = a ^ b
    s = 1
    p = max(a, b)
    d = bin(a).count('1')
    e = 1
    while e <= p:
        if e & a:
            d -= 1
        if d & 1 and e & b:
            s = -s
        e *= 2
    return c, s


def _sign_tables():
    """IDX[i,c] = i^c ;  SGN[i,c] = rs[i, i^c]."""
    if "t" in _tables_cache:
        return _tables_cache["t"]
    rs = np.zeros((S, S), dtype=np.float32)
    for i in range(S):
        for j in range(S):
            _, s = _blade_combine(i, j)
            rs[i, j] = s
    ii = np.arange(S)[:, None]
    cc = np.arange(S)[None, :]
    idx = ii ^ cc                      # [S, S] int
    sgn = rs[ii, idx]                  # [S, S] = rs[i, i^c]
    _tables_cache["t"] = (idx, sgn)
    return idx, sgn


# ---------------------------------------------------------------------------
# Device graph
# ---------------------------------------------------------------------------
_graph_cache = {}


def _build_graph(with_bias: bool):
    import concourse.bacc as bacc
    import concourse.mybir as mybir
    from concourse import tile

    nc = bacc.Bacc(None)
    f32 = mybir.dt.float32
    bf16 = mybir.dt.bfloat16

    at = nc.declare_dram_parameter("at", [S, BPC], bf16, isOutput=False)
    kic = nc.declare_dram_parameter("kic", [S, KC], bf16, isOutput=False)
    if with_bias:
        biasb = nc.declare_dram_parameter("biasb", [1, KC], bf16, isOutput=False)
    out = nc.declare_dram_parameter("out", [BPC, KC], f32, isOutput=True)

    NB = BPC // 128    # 16 row tiles
    NN = KC // 512     # 8 psum-bank column chunks

    with tile.TileContext(nc) as tc:
        with (
            tc.tile_pool(name="const", bufs=1) as cpool,
            tc.tile_pool(name="outp", bufs=3) as opool,
            tc.tile_pool(name="ps", bufs=8, space="PSUM") as ppool,
        ):
            at_sb = cpool.tile([128, 2, BPC], bf16)
            kic_sb = cpool.tile([128, 2, KC], bf16)
            nc.sync.dma_start(at_sb[:, 0, :], at[0:128, :])
            nc.sync.dma_start(at_sb[:, 1, :], at[128:256, :])
            nc.sync.dma_start(kic_sb[:, 0, :], kic[0:128, :])
            nc.sync.dma_start(kic_sb[:, 1, :], kic[128:256, :])
            if with_bias:
                bias_sb = cpool.tile([1, KC], bf16)
                ones_sb = cpool.tile([1, 128], bf16)
                nc.sync.dma_start(bias_sb[:], biasb[:])
                nc.vector.memset(ones_sb[:], 1.0)

            for bt in range(NB):
                out_sb = opool.tile([128, KC], f32)
                for nch in range(NN):
                    ps = ppool.tile([128, 512], f32)
                    ns = slice(nch * 512, (nch + 1) * 512)
                    if with_bias:
                        nc.tensor.matmul(
                            ps[:], ones_sb[:, :], bias_sb[:, ns],
                            start=True, stop=False,
                        )
                    for ic in range(2):
                        nc.tensor.matmul(
                            ps[:],
                            at_sb[:, ic, bt * 128:(bt + 1) * 128],
                            kic_sb[:, ic, ns],
                            start=(ic == 0 and not with_bias),
                            stop=(ic == 1),
                        )
                    # drain PSUM -> SBUF, alternating engines
                    if nch % 2 == 0:
                        nc.scalar.copy(out_sb[:, ns], ps[:])
                    else:
                        nc.vector.tensor_copy(out_sb[:, ns], ps[:])
                nc.sync.dma_start(out[bt * 128:(bt + 1) * 128, :], out_sb[:])

    nc.compile()
    return nc


def _get_graph(with_bias: bool):
    if with_bias not in _graph_cache:
        _graph_cache[with_bias] = _build_graph(with_bias)
    return _graph_cache[with_bias]


# ---------------------------------------------------------------------------
# Entry point
# ---------------------------------------------------------------------------

def kernel(input, weight, bias, _trace=False):
    from concourse.bass_utils import run_bass_kernel_spmd

    input = np.asarray(input)
    weight = np.asarray(weight)
    bias = np.asarray(bias)

    idx, sgn = _sign_tables()

    # kic[i, k*S + c] = weight[k, i^c] * rs[i, i^c]
    kic = (weight[:, idx] * sgn[None, :, :]).transpose(1, 0, 2).reshape(S, KC)
    kic_bf = np.ascontiguousarray(kic.astype(_BF16))

    with_bias = bool(np.any(bias != 0.0))
    nc = _get_graph(with_bias)

    in_maps = []
    for c in range(NCORES):
        shard = input[c * BPC:(c + 1) * BPC, :]          # [BPC, S]
        at = np.ascontiguousarray(shard.T.astype(_BF16))  # [S, BPC]
        m = {"at": at, "kic": kic_bf}
        if with_bias:
            m["biasb"] = np.ascontiguousarray(
                np.tile(bias, K).reshape(1, KC).astype(_BF16))
        in_maps.append(m)

    res = run_bass_kernel_spmd(
        nc, in_maps, core_ids=list(range(NCORES)), trace=_trace,
    )
    outs = [res.results[c]["out"] for c in range(NCORES)]
    full = np.concatenate(outs, axis=0).reshape(BATCH, K, S)
    if _trace:
        kernel.last_exec_time_ns = res.exec_time_ns
        kernel.last_profile = res
    return full


# revision 5
# speedup vs baseline: 1.2936x; 1.2936x over previous
"""Trainium2 Bass kernel for nn_CFLiner (Clifford-algebra linear layer).

Math: out[b,k,c] = sum_{i,j} input[b,i] * weight[k,j] * M[i,j,c] + bias[c]
where M[i,j,c] = rs[i,j] * [c == i^j] is the Cl(8,0) blade-product tensor.
Since rk[i,j] = i XOR j, folding weight into M is a signed gather:
    kic[i, (k,c)] = weight[k, i^c] * rs[i, i^c]
and the heavy op is a dense matmul  out[b, (k,c)] = input[b,:] @ kic[:, (k,c)].

Strategy: pure data parallelism over 8 NeuronCores (batch 16384 -> 2048/core).
Host prepares (outside HW timing): A^T shards + kic in bf16. Device runs the
[2048,256]x[256,4096] matmul in bf16 on the PE, drains PSUM via ScalarE/VectorE,
DMAs the f32 output (32MB/core) back -- memory-bound on the output write.
"""

import sys
import numpy as np

for _p in ("/opt/trn_rl_repo",):
    if _p not in sys.path:
        sys.path.append(_p)

import ml_dtypes

BATCH = 16384
S = 256          # blade dimension
K = 16           # out channels
NCORES = 8
BPC = BATCH // NCORES   # 2048 rows per core
KC = K * S              # 4096 output columns (k*256 + c)

_BF16 = ml_dtypes.bfloat16

# ---------------------------------------------------------------------------
# Compile-time constant tables (blade-product structure of Cl(8,0))
# ---------------------------------------------------------------------------
_tables_cache = {}


def _blade_combine(a, b):
    if a == 0:
        return b, 1
    if b == 0:
        return a, 1
    c = a ^ b
    s = 1
    p = max(a, b)
    d = bin(a).count('1')
    e = 1
    while e <= p:
        if e & a:
            d -= 1
        if d & 1 and e & b:
            s = -s
        e *= 2
    return c, s


def _sign_tables():
    """IDX[i,c] = i^c ;  SGN[i,c] = rs[i, i^c]."""
    if "t" in _tables_cache:
        return _tables_cache["t"]
    rs = np.zeros((S, S), dtype=np.float32)
    for i in range(S):
        for j in range(S):
            _, s = _blade_combine(i, j)
            rs[i, j] = s
    ii = np.arange(S)[:, None]
    cc = np.arange(S)[None, :]
    idx = ii ^ cc                      # [S, S] int
    sgn = rs[ii, idx]                  # [S, S] = rs[i, i^c]
    _tables_cache["t"] = (idx, sgn)
    return idx, sgn


# ---------------------------------------------------------------------------
# Device graph
# ---------------------------------------------------------------------------
_graph_cache = {}


def _build_graph(with_bias: bool):
    import concourse.bacc as bacc
    import concourse.mybir as mybir
    from concourse import tile

    nc = bacc.Bacc(None)
    f32 = mybir.dt.float32
    bf16 = mybir.dt.bfloat16

    at = nc.declare_dram_parameter("at", [S, BPC], bf16, isOutput=False)
    kic = nc.declare_dram_parameter("kic", [S, KC], bf16, isOutput=False)
    if with_bias:
        biasb = nc.declare_dram_parameter("biasb", [1, KC], bf16, isOutput=False)
    out = nc.declare_dram_parameter("out", [BPC, KC], bf16, isOutput=True)

    NB = BPC // 128    # 16 row tiles
    NN = KC // 512     # 8 psum-bank column chunks

    with tile.TileContext(nc) as tc:
        with (
            tc.tile_pool(name="const", bufs=1) as cpool,
            tc.tile_pool(name="outp", bufs=3) as opool,
            tc.tile_pool(name="ps", bufs=8, space="PSUM") as ppool,
        ):
            at_sb = cpool.tile([128, 2, BPC], bf16)
            kic_sb = cpool.tile([128, 2, KC], bf16)
            nc.sync.dma_start(at_sb[:, 0, :], at[0:128, :])
            nc.sync.dma_start(at_sb[:, 1, :], at[128:256, :])
            nc.sync.dma_start(kic_sb[:, 0, :], kic[0:128, :])
            nc.sync.dma_start(kic_sb[:, 1, :], kic[128:256, :])
            if with_bias:
                bias_sb = cpool.tile([1, KC], bf16)
                ones_sb = cpool.tile([1, 128], bf16)
                nc.sync.dma_start(bias_sb[:], biasb[:])
                nc.vector.memset(ones_sb[:], 1.0)

            for bt in range(NB):
                out_sb = opool.tile([128, KC], bf16)
                for nch in range(NN):
                    ps = ppool.tile([128, 512], f32)
                    ns = slice(nch * 512, (nch + 1) * 512)
                    if with_bias:
                        nc.tensor.matmul(
                            ps[:], ones_sb[:, :], bias_sb[:, ns],
                            start=True, stop=False,
                        )
                    for ic in range(2):
                        nc.tensor.matmul(
                            ps[:],
                            at_sb[:, ic, bt * 128:(bt + 1) * 128],
                            kic_sb[:, ic, ns],
                            start=(ic == 0 and not with_bias),
                            stop=(ic == 1),
                        )
                    # drain PSUM -> SBUF, alternating engines
                    if nch % 2 == 0:
                        nc.scalar.copy(out_sb[:, ns], ps[:])
                    else:
                        nc.vector.tensor_copy(out_sb[:, ns], ps[:])
                nc.sync.dma_start(out[bt * 128:(bt + 1) * 128, :], out_sb[:])

    nc.compile()
    return nc


def _get_graph(with_bias: bool):
    if with_bias not in _graph_cache:
        _graph_cache[with_bias] = _build_graph(with_bias)
    return _graph_cache[with_bias]


# ---------------------------------------------------------------------------
# Entry point
# ---------------------------------------------------------------------------

def kernel(input, weight, bias, _trace=False):
    from concourse.bass_utils import run_bass_kernel_spmd

    input = np.asarray(input)
    weight = np.asarray(weight)
    bias = np.asarray(bias)

    idx, sgn = _sign_tables()

    # kic[i, k*S + c] = weight[k, i^c] * rs[i, i^c]
    kic = (weight[:, idx] * sgn[None, :, :]).transpose(1, 0, 2).reshape(S, KC)
    kic_bf = np.ascontiguousarray(kic.astype(_BF16))

    with_bias = bool(np.any(bias != 0.0))
    nc = _get_graph(with_bias)

    in_maps = []
    for c in range(NCORES):
        shard = input[c * BPC:(c + 1) * BPC, :]          # [BPC, S]
        at = np.ascontiguousarray(shard.T.astype(_BF16))  # [S, BPC]
        m = {"at": at, "kic": kic_bf}
        if with_bias:
            m["biasb"] = np.ascontiguousarray(
                np.tile(bias, K).reshape(1, KC).astype(_BF16))
        in_maps.append(m)

    res = run_bass_kernel_spmd(
        nc, in_maps, core_ids=list(range(NCORES)), trace=_trace,
    )
    outs = [res.results[c]["out"].astype(np.float32) for c in range(NCORES)]
    full = np.concatenate(outs, axis=0).reshape(BATCH, K, S)
    if _trace:
        kernel.last_exec_time_ns = res.exec_time_ns
        kernel.last_profile = res
    return full
